# revision 78
# baseline (speedup 1.0000x reference)
"""LISTA (learned ISTA) sparse-coding forward pass on 8 Trainium2 NeuronCores.

Problem: I [4,1,192,192] -> im2col(9x9) -> 24 soft-thresholded iterations over
64 filters -> decode -> col2im overlap-add average -> [4,1,192,192].

Sharding: 8 cores = 4 images x 2 position-row halves (92 rows of 184 positions
each). Each core computes its full LISTA pipeline plus the col2im partial sums
for its 100-row output slab; the host merges the 8-row seams between the two
slabs of each image and divides by the overlap counts (pure unshard glue).

Algebra used (exact rewrites of the reference up to fp assoc.):
  - mean-subtraction folded into encoder:  c = WAc @ I_col,
      WAc = WA - rowmean(WA)  (since mean_patch = (1/81) * ones^T I_col)
  - iteration fused:  gamma_{t+1} = soft(S @ gamma_t + c),  S = I - WA@WD
  - mean add-back is separable: col2im_avg(mean x ones81) = 9-tap box filters
    (rows then cols) applied twice to the input slab; computed with a handful
    of tiny matmuls in the otherwise-idle pre-encode window, added at the end.
  - decode therefore only needs out_all' = WW @ gamma (one stationary / half).

Per-iteration structure (the hot loop, per core: 128 partitions = 2 halves x
64 filters, 8464 position columns):
  PE:  y = Id@c + Sbd@gam into PSUM in five supersteps (4x2048 + 272)
  ACT: copies supersteps 0-3 PSUM->SBUF (4 big ACTIVATEs)
  DVE: casts the 272 tail, then clip (tensor_scalar 4x) + subtract
       (tensor_tensor 2x) to form gam = soft(y).
col2im uses 4 rotating PSUM accumulator slots so its 81 shift-matmuls
pipeline instead of serializing on one bank.
"""

import contextlib
import numpy as np

# ---------------------------------------------------------------- constants
B, H, Wimg = 4, 192, 192
K = 9
F = 64
NCH = K * K  # 81
HO = H - K + 1  # 184
WO = Wimg - K + 1  # 184
UNF = 24
N_CORES = 8

ROWS = HO // 2  # 92 position rows per core
SLAB = ROWS + K - 1  # 100 image/output rows per core
NPOS = ROWS * WO  # 16928 positions per core
HALFR = ROWS // 2  # 46 rows per block-diag half
HALF = HALFR * WO  # 8464 columns per half

# iteration chunking: 512-col psum chunks, paired into 1024-col supersteps
CH = 512
CHUNKS = [(i * CH, min((i + 1) * CH, HALF)) for i in range((HALF + CH - 1) // CH)]
SUPERS = [CHUNKS[i: i + 2] for i in range(0, len(CHUNKS), 2)]

DCH = 2 * WO  # decode group = 2 position rows = 368 columns
NGRP = NPOS // DCH  # 46 groups (23 per half)
GPT = 2  # groups per decode psum tile (one per 512-col bank slot)

IMW_IM2COL = K * SLAB * WO  # 165600
IMW_TOT = IMW_IM2COL + SLAB * Wimg  # + raw slab [100,192]

# weight blob layout: (name, partitions, cols) — bf16
BLOB_SPEC = [
    ("wac", NCH, F), ("wacp", NCH, 128), ("sbd", 128, 128), ("nsbd", 128, 128),
    ("id128", 128, 128), ("wwa", 128, NCH), ("wwb", 128, NCH),
    ("eshb", ROWS, K * SLAB), ("bnd1", SLAB, ROWS), ("bnd2", ROWS, SLAB),
]
BLOBC = sum(nf for _, _, nf in BLOB_SPEC)

_STATE = {}


def _split_multi_waits(nc, mybir):
    """This walrus build supports a single sync-wait slot per instruction.
    Move extra waits onto preceding same-engine no-ops (same semantics:
    program order on one engine; all waits clear before the instruction)."""
    cnt = 0
    for fn in nc.m.functions:
        for bb in fn.blocks:
            insts = bb.instructions
            need = False
            for ins in insts:
                si = ins.sync_info
                if si is not None and si.on_wait is not None and len(si.on_wait) > 1:
                    need = True
                    break
            if not need:
                continue
            out = []
            for ins in insts:
                si = ins.sync_info
                if si is not None and si.on_wait is not None and len(si.on_wait) > 1:
                    waits = list(si.on_wait)
                    for w in waits[:-1]:
                        cnt += 1
                        nop = mybir.InstNoOp(name=f"wsplit-{cnt}", ins=[], outs=[])
                        nop.engine = ins.engine
                        nop.sync_info = mybir.SyncInfo(on_wait=[w], on_update=[])
                        out.append(nop)
                    ins.sync_info = mybir.SyncInfo(
                        on_wait=[waits[-1]], on_update=list(si.on_update or [])
                    )
                out.append(ins)
            bb.instructions = out
    return cnt


def _build():
    import concourse.bass as bass
    import concourse.mybir as mybir
    import concourse.tile as tile

    f32 = mybir.dt.float32
    bf16 = mybir.dt.bfloat16
    f8 = mybir.dt.float8e4
    Alu = mybir.AluOpType
    Act = mybir.ActivationFunctionType

    nc = bass.Bass("TRN2", target_bir_lowering=False, debug=False)

    imgw = nc.dram_tensor("imgw", [IMW_TOT], bf16, kind="ExternalInput").ap()
    blob_d = nc.dram_tensor("blob", [128, BLOBC], bf16, kind="ExternalInput").ap()
    lams_d = nc.dram_tensor("lams", [128, 2], f32, kind="ExternalInput").ap()
    out_d = nc.dram_tensor("out", [SLAB, Wimg], f32, kind="ExternalOutput").ap()
    # row-range-split obuf tensors: first-half col2im gathers start
    # mid-decode instead of waiting for a full-queue drain at the end
    RSPL = [0, 48, ROWS]
    obufs = [nc.dram_tensor(f"obuf{p}", [NCH * (RSPL[p + 1] - RSPL[p]) * WO],
                            bf16, kind="Internal").ap() for p in range(2)]

    with tile.TileContext(nc) as tc:
        with contextlib.ExitStack() as ctx:
            wpool = ctx.enter_context(tc.tile_pool(name="w", bufs=1))
            big = ctx.enter_context(tc.tile_pool(name="big", bufs=1))
            pp = ctx.enter_context(tc.tile_pool(name="ps", bufs=4, space="PSUM"))
            stg = ctx.enter_context(tc.tile_pool(name="stg", bufs=12))

            blob = wpool.tile([128, BLOBC], bf16)
            lams = wpool.tile([128, 2], f32)
            o = {}
            col = 0
            for name, np_, nf in BLOB_SPEC:
                o[name] = (np_, col, nf)
                col += nf

            def bl(name):
                np_, c0, nf = o[name]
                return blob[0:np_, c0:c0 + nf]

            wac = bl("wac"); wacp = bl("wacp"); sbd = bl("sbd")
            nsbd = bl("nsbd")
            id128 = bl("id128"); wwa = bl("wwa"); wwb = bl("wwb")
            eshb = bl("eshb"); bnd1 = bl("bnd1"); bnd2 = bl("bnd2")
            id100 = blob[0:SLAB, o["id128"][1]:o["id128"][1] + SLAB]
            lam = lams[:, 0:1]
            nlam = lams[:, 1:2]

            # persistent SBUF state
            icol = big.tile([NCH, NPOS], bf16, tag="icol")
            c = big.tile([128, HALF], bf16)
            gam = big.tile([128, HALF], bf16)
            yt = big.tile([128, HALF], bf16)
            zt = big.tile([128, HALF], bf16)
            slab = big.tile([SLAB, Wimg], bf16)
            t2m = big.tile([SLAB, Wimg], bf16)  # mean-path output grid term
            ms = big.tile([SLAB, 1024], bf16)   # mean-path scratch columns
            s1 = ms[:, 0:184]
            mrow = ms[0:ROWS, 192:376]
            t1 = ms[:, 384:568]
            ta = ms[:, 576:760]
            tb = ms[:, 768:960]

            # ---- input DMAs: slab first (mean path), then weights, then icol
            nc.sync.dma_start(
                slab[:], bass.AP(imgw.tensor, IMW_IM2COL, [[Wimg, SLAB], [1, Wimg]]))
            nc.scalar.dma_start(blob[:], blob_d)
            nc.scalar.dma_start(lams[:], lams_d)
            # im2col: host supplies img_w[kw] = slab[:, kw:kw+WO]; each channel
            # (kh, kw) = img_w[kw][kh:kh+ROWS] is one contiguous run.
            # 8 pieces ordered so early position rows land first (encode overlap).
            engs = (nc.sync, nc.gpsimd, nc.scalar)
            ei = 0
            for q in range(2):
                for hh in range(2):
                    r0 = hh * HALFR + q * (HALFR // 2)
                    r1 = hh * HALFR + (q + 1) * (HALFR // 2)
                    engs[ei % 3].dma_start(
                        icol[:, r0 * WO:r1 * WO],
                        bass.AP(imgw.tensor, r0 * WO,
                                [[WO, K], [SLAB * WO, K], [1, (r1 - r0) * WO]]))
                    ei += 1

            # ---- mean pathway (separable 9x9 box filters), pre-encode window.
            # M2[y,x] = sum_{kh,kw valid} mean[y-kh, x-kw],
            # mean[r,w] = (1/81) sum_{kh,kw} slab[r+kh, w+kw]
            # (idle pre-encode window: serialization of these tiny matmuls is
            # harmless, and every read element is written at least once)
            M1 = pp.tile([128, 1024], f32, tag="ps")
            PC = M1[0:ROWS, 512:696]
            for kw in range(K):
                # S1[w] = sum_kw slab[w+kw]: fixed out window, sliding rhs
                nc.tensor.matmul(M1[0:SLAB, 0:WO], id100,
                                 slab[:, kw:kw + WO],
                                 start=(kw == 0), stop=(kw == K - 1))
            nc.scalar.copy(ta[:, 0:184], M1[0:SLAB, 0:184])
            nc.tensor.matmul(PC, bnd1, ta[:, 0:184], start=True, stop=True)
            nc.scalar.copy(mrow, PC)  # bnd1 carries the 1/81 scale
            M2 = pp.tile([128, 1024], f32, tag="ps")
            nc.tensor.matmul(M2[0:SLAB, 512:696], bnd2, mrow,
                             start=True, stop=True)
            nc.scalar.copy(t1, M2[0:SLAB, 512:696])
            for kw in range(K):
                nc.tensor.matmul(M2[0:SLAB, kw:kw + 184], id100, t1,
                                 start=(kw == 0), stop=(kw == K - 1))
            nc.scalar.copy(t2m[:], M2[0:SLAB, 0:192])

            # ---- encode: c = WAc @ I_col for both halves (B via col-tile 64)
            for si, sup in enumerate(SUPERS):
                ps = pp.tile([128, 1024], f32, tag="ps")
                c0s, c1s = sup[0][0], sup[-1][1]
                for jj, (c0, c1) in enumerate(sup):
                    n = c1 - c0
                    nc.tensor.matmul(ps[0:128, jj * CH: jj * CH + n], wacp,
                                     icol[:, HALF + c0: HALF + c1],
                                     start=True, stop=True)
                    nc.tensor.matmul(ps[0:F, jj * CH: jj * CH + n], wac,
                                     icol[:, c0:c1], start=True, stop=True)
                span = c1s - c0s
                nc.scalar.copy(c[:, c0s:c1s], ps[:, 0:span])
                # gamma0 = c - clip(c) directly after each superstep's c lands
                nc.vector.tensor_scalar(zt[:, c0s:c1s], c[:, c0s:c1s],
                                        lam, nlam, Alu.min, Alu.max)
                nc.vector.tensor_tensor(gam[:, c0s:c1s], c[:, c0s:c1s],
                                        zt[:, c0s:c1s], Alu.subtract)
            # scheme-Z state init: y_0 = c (zt already holds clip(c))
            nc.vector.tensor_copy(yt[:, 7 * 1024:HALF], c[:, 7 * 1024:HALF])

            # ---- 23 fused iterations: y = Id@c + S@gam; gam' = y - clip(y)
            # Columns >= ZC (supersteps 7-8) use (y, z=clip(y)) state instead
            # of gam: the PE adds a third pass (-S@z), which deletes the DVE
            # subtract there; the freed DVE capacity casts cols >= ACT_END so
            # ACT streams less. The last iteration reverts to gam everywhere
            # (decode reads gam). soft(y) = y - clip(y) makes this exact.
            ZC = 7 * 1024  # scheme-Z columns [7168:8464]
            ACT_END = ZC + CH  # ACT copies [0:7680], DVE casts [7680:8464]
            for _t in range(UNF - 1):
                lastit = _t == UNF - 2
                for sp in range(0, len(SUPERS), 2):
                    pair = SUPERS[sp:sp + 2]
                    p0 = pair[0][0][0]
                    off = 0
                    for sup in pair:
                        ps = pp.tile([128, 1024], f32, tag="ps")
                        c0s, c1s = sup[0][0], sup[-1][1]
                        for jj, (c0, c1) in enumerate(sup):
                            nc.tensor.matmul(ps[:, jj * CH: jj * CH + (c1 - c0)],
                                             id128, c[:, c0:c1],
                                             start=True, stop=False)
                        for jj, (c0, c1) in enumerate(sup):
                            if c0 >= ZC:
                                nc.tensor.matmul(
                                    ps[:, jj * CH: jj * CH + (c1 - c0)],
                                    sbd, yt[:, c0:c1], start=False, stop=False)
                                nc.tensor.matmul(
                                    ps[:, jj * CH: jj * CH + (c1 - c0)],
                                    nsbd, zt[:, c0:c1], start=False, stop=True)
                            else:
                                nc.tensor.matmul(
                                    ps[:, jj * CH: jj * CH + (c1 - c0)],
                                    sbd, gam[:, c0:c1], start=False, stop=True)
                        span = c1s - c0s
                        if c0s >= ACT_END:
                            nc.vector.tensor_copy(yt[:, c0s:c1s], ps[:, 0:span])
                        elif c1s > ACT_END:
                            na = ACT_END - c0s
                            nc.scalar.copy(yt[:, c0s:ACT_END], ps[:, 0:na])
                            nc.vector.tensor_copy(yt[:, ACT_END:c1s],
                                                  ps[:, na:span])
                        else:
                            nc.scalar.copy(yt[:, c0s:c1s], ps[:, 0:span])
                        off += span
                    if sp < 6:
                        nc.vector.tensor_scalar(zt[:, p0:p0 + off],
                                                yt[:, p0:p0 + off],
                                                lam, nlam, Alu.min, Alu.max)
                        if sp < 4:
                            nc.vector.tensor_tensor(gam[:, p0:p0 + off],
                                                    yt[:, p0:p0 + off],
                                                    zt[:, p0:p0 + off],
                                                    Alu.subtract)
                    elif sp == 8:
                        # merged clip over [6144:8464] and merged subtract
                        # over [4096:7168]: the Z-region has no subtract, and
                        # both outputs aren't read until well into the next
                        # iteration, so two wide ops replace four
                        nc.vector.tensor_scalar(zt[:, 6144:HALF],
                                                yt[:, 6144:HALF],
                                                lam, nlam, Alu.min, Alu.max)
                        tte = HALF if lastit else ZC
                        nc.vector.tensor_tensor(gam[:, 4096:tte],
                                                yt[:, 4096:tte],
                                                zt[:, 4096:tte], Alu.subtract)
                # pad the psum-pool rotation to 12 fills/iteration so every
                # iteration starts at the same (measured-fastest) buffer
                # phase; the 1x1 matmuls cost ~60ns each of idle PE slack
                for _dmy in range(3):
                    dmy = pp.tile([128, 1024], f32, tag="ps")
                    nc.tensor.matmul(dmy[0:1, 0:1], id128[0:1, 0:1],
                                     c[0:1, 0:1], start=True, stop=True)

            # ---- decode: out_all' = WW@gam per half, stream to HBM (obuf).
            # 46 two-row groups, 2 per psum tile in 512-col bank slots; one
            # strided extraction per tile alternating ACT/DVE.
            ntile = (NGRP + GPT - 1) // GPT
            stall = big.tile([ROWS, NCH * WO], bf16, tag="icol")
            for t in range(ntile):
                g0 = t * GPT
                ng = min(GPT, NGRP - g0)
                ps = pp.tile([128, 1024], f32, tag="ps")
                for j in range(ng):
                    g = g0 + j
                    half, gl = g // (NGRP // 2), g % (NGRP // 2)
                    ww = wwb if half else wwa
                    cc = gl * DCH
                    nc.tensor.matmul(ps[0:NCH, j * 512:j * 512 + DCH], ww,
                                     gam[:, cc:cc + DCH], start=True, stop=True)
                yd = stg.tile([NCH, GPT * DCH], bf16, tag="yd")
                src = ps.rearrange("p (g x) -> p g x", g=2)[0:NCH, 0:ng, 0:DCH]
                dst = yd.rearrange("p (g x) -> p g x", g=GPT)[:, 0:ng, :]
                if t % 2 == 0:
                    nc.scalar.copy(dst, src)
                else:
                    nc.vector.tensor_copy(dst, src)
                # each group = 2 position rows; rows are globally contiguous
                r0 = 2 * g0
                nrows = 2 * ng
                part = 0 if r0 < RSPL[1] else 1
                ob, rb = obufs[part], r0 - RSPL[part]
                dmadst = bass.AP(ob.tensor, rb * NCH * WO,
                                 [[WO, NCH], [NCH * WO, nrows], [1, WO]])
                eng = (nc.sync, nc.gpsimd, nc.scalar)[t % 3]
                eng.dma_start(dmadst, yd[:, 0:ng * DCH])
                if r0 + nrows == RSPL[1]:
                    # first-half rows fully written: start their gathers now
                    for kh in range(K):
                        geng = (nc.sync, nc.gpsimd)[kh % 2]
                        geng.dma_start(
                            stall[0:RSPL[1], kh * K * WO:(kh + 1) * K * WO],
                            bass.AP(obufs[0].tensor, kh * K * WO,
                                    [[NCH * WO, RSPL[1]], [1, K * WO]]))

            # ---- col2im: contiguous gathers (reuse icol's SBUF slot), then
            # shift-matmuls into 4 rotating PSUM bank slots (no serialization),
            # merged with the mean term at the end.
            for kh in range(K):
                eng = (nc.sync, nc.gpsimd)[(kh + 1) % 2]
                eng.dma_start(
                    stall[RSPL[1]:ROWS, kh * K * WO:(kh + 1) * K * WO],
                    bass.AP(obufs[1].tensor, kh * K * WO,
                            [[NCH * WO, ROWS - RSPL[1]], [1, K * WO]]))
            opsA = pp.tile([128, 1024], f32, tag="ps")
            opsB = pp.tile([128, 1024], f32, tag="ps")
            tiles = [opsA, opsA, opsB, opsB]
            bases = [0, 512, 0, 512]
            started = [False] * 4
            nmm = [0] * 4
            for i in range(NCH):
                nmm[i % 4] += 1
            done = [0] * 4
            for kh in range(K):
                lhs = eshb[:, kh * SLAB:(kh + 1) * SLAB]
                for kw in range(K):
                    i = kh * K + kw
                    sl = i % 4
                    done[sl] += 1
                    nc.tensor.matmul(
                        tiles[sl][0:SLAB, bases[sl] + kw:bases[sl] + kw + WO],
                        lhs, stall[:, i * WO:(i + 1) * WO],
                        start=not started[sl], stop=(done[sl] == nmm[sl]))
                    started[sl] = True
            # merge 4 slots + mean term -> f32 out
            q0 = stg.tile([SLAB, Wimg], bf16, tag="q0")
            q1 = stg.tile([SLAB, Wimg], bf16, tag="q1")
            u0 = stg.tile([SLAB, Wimg], bf16, tag="u0")
            u1 = stg.tile([SLAB, Wimg], bf16, tag="u1")
            acc = stg.tile([SLAB, Wimg], f32, tag="acc")
            nc.scalar.copy(q0[:], opsA[0:SLAB, 0:192])
            nc.vector.tensor_copy(q1[:], opsA[0:SLAB, 512:704])
            nc.vector.tensor_tensor(u0[:], q0[:], opsB[0:SLAB, 0:192], Alu.add)
            nc.vector.tensor_tensor(u1[:], q1[:], opsB[0:SLAB, 512:704], Alu.add)
            nc.vector.tensor_tensor(u0[:], u0[:], u1[:], Alu.add)
            nc.vector.tensor_tensor(u1[:], u0[:], t2m[:], Alu.add)
            nc.vector.tensor_copy(acc[:], u1[:])
            nc.sync.dma_start(out_d, acc[:])

    _split_multi_waits(nc, mybir)
    return nc


def _get_nc():
    if "nc" not in _STATE:
        _STATE["nc"] = _build()
    return _STATE["nc"]


def _make_in_maps(I, WA, WD, WW, lmbda):
    import ml_dtypes  # noqa: F401
    I = np.ascontiguousarray(np.asarray(I, np.float32))
    WA = np.asarray(WA, np.float32)
    WD = np.asarray(WD, np.float32)
    WW = np.asarray(WW, np.float32)
    lam = np.asarray(lmbda, np.float32).reshape(F)
    assert I.shape == (B, 1, H, Wimg)

    WAc = (WA - WA.mean(axis=1, keepdims=True)).astype(np.float32)  # [64,81]
    S = (np.eye(F, dtype=np.float32) - WA @ WD).astype(np.float32)  # [64,64]
    sbd = np.zeros((128, 128), np.float32)
    sbd[0:F, 0:F] = S.T
    sbd[F:128, F:128] = S.T
    id128 = np.eye(128, dtype=np.float32)
    wacp = np.zeros((81, 128), np.float32)
    wacp[:, F:128] = WAc.T
    wwa = np.zeros((128, 81), np.float32)
    wwa[0:F, :] = WW.T
    wwb = np.zeros((128, 81), np.float32)
    wwb[F:128, :] = WW.T
    lam128 = np.concatenate([lam, lam]).reshape(128, 1).astype(np.float32)
    esh = np.zeros((ROWS, K * SLAB), np.float32)  # lhsT per kh: E[r, y]=1 iff y=r+kh
    for kh in range(K):
        for rr in range(ROWS):
            esh[rr, kh * SLAB + rr + kh] = 1.0
    bnd1 = np.zeros((SLAB, ROWS), np.float32)  # S2[r] = sum_{p=r..r+8} S1[p] / 81
    for p in range(SLAB):
        for r in range(ROWS):
            if r <= p <= r + 8:
                bnd1[p, r] = 1.0 / NCH
    bnd2 = np.zeros((ROWS, SLAB), np.float32)  # T1[y] = sum_{r=y-8..y} m[r]
    for r in range(ROWS):
        for y in range(SLAB):
            if y - 8 <= r <= y:
                bnd2[r, y] = 1.0
    vals = {"wac": WAc.T, "wacp": wacp, "sbd": sbd, "nsbd": -sbd,
            "id128": id128, "wwa": wwa, "wwb": wwb, "eshb": esh,
            "bnd1": bnd1, "bnd2": bnd2}
    blob = np.zeros((128, BLOBC), np.float32)
    col = 0
    for name, np_, nf in BLOB_SPEC:
        v = np.asarray(vals[name], np.float32)
        assert v.shape == (np_, nf), (name, v.shape)
        blob[0:np_, col:col + nf] = v
        col += nf
    lams = np.concatenate([lam128, -lam128], axis=1).astype(np.float32)

    shared = {"blob": blob.astype(ml_dtypes.bfloat16), "lams": lams}
    in_maps = []
    for core in range(N_CORES):
        b, h = core // 2, core % 2
        r0 = h * ROWS
        slab = I[b, 0, r0:r0 + SLAB, :]
        imgw = np.stack([slab[:, kw:kw + WO] for kw in range(K)], axis=0)
        full = np.concatenate([
            np.ascontiguousarray(imgw).reshape(-1),
            np.ascontiguousarray(slab).reshape(-1)])
        in_maps.append({"imgw": full.astype(ml_dtypes.bfloat16), **shared})
    return in_maps


def _unshard(results):
    cnt = np.zeros((H, Wimg), np.float32)
    for kh in range(K):
        for kw in range(K):
            cnt[kh:kh + HO, kw:kw + WO] += 1.0
    out = np.zeros((B, 1, H, Wimg), np.float32)
    for b in range(B):
        acc = np.zeros((H, Wimg), np.float32)
        acc[0:SLAB, :] += results[2 * b]["out"]
        acc[ROWS:ROWS + SLAB, :] += results[2 * b + 1]["out"]
        out[b, 0] = acc / cnt
    return out


def kernel(I, WA, WD, WW, lmbda, kernel_size=9, stride=1, unfoldings=24, **_kw):
    from concourse import bass_utils

    assert int(kernel_size) == K and int(stride) == 1 and int(unfoldings) == UNF
    in_maps = _make_in_maps(I, WA, WD, WW, lmbda)
    nc = _get_nc()
    last = None
    for _attempt in range(3):
        try:
            res = bass_utils.run_bass_kernel_spmd(
                nc, in_maps, core_ids=list(range(N_CORES)))
            return _unshard(res.results)
        except Exception as e:  # transient NRT device errors: retry
            last = e
    raise last


# revision 79
# speedup vs baseline: 1.0114x; 1.0114x over previous
"""LISTA (learned ISTA) sparse-coding forward pass on 8 Trainium2 NeuronCores.

Problem: I [4,1,192,192] -> im2col(9x9) -> 24 soft-thresholded iterations over
64 filters -> decode -> col2im overlap-add average -> [4,1,192,192].

Sharding: 8 cores = 4 images x 2 position-row halves (92 rows of 184 positions
each). Each core computes its full LISTA pipeline plus the col2im partial sums
for its 100-row output slab; the host merges the 8-row seams between the two
slabs of each image and divides by the overlap counts (pure unshard glue).

Algebra used (exact rewrites of the reference up to fp assoc.):
  - mean-subtraction folded into encoder:  c = WAc @ I_col,
      WAc = WA - rowmean(WA)  (since mean_patch = (1/81) * ones^T I_col)
  - iteration fused:  gamma_{t+1} = soft(S @ gamma_t + c),  S = I - WA@WD
  - mean add-back is separable: col2im_avg(mean x ones81) = 9-tap box filters
    (rows then cols) applied twice to the input slab; computed with a handful
    of tiny matmuls in the otherwise-idle pre-encode window, added at the end.
  - decode therefore only needs out_all' = WW @ gamma (one stationary / half).

Per-iteration structure (the hot loop, per core: 128 partitions = 2 halves x
64 filters, 8464 position columns):
  PE:  y = Id@c + Sbd@gam into PSUM in five supersteps (4x2048 + 272)
  ACT: copies supersteps 0-3 PSUM->SBUF (4 big ACTIVATEs)
  DVE: casts the 272 tail, then clip (tensor_scalar 4x) + subtract
       (tensor_tensor 2x) to form gam = soft(y).
col2im uses 4 rotating PSUM accumulator slots so its 81 shift-matmuls
pipeline instead of serializing on one bank.
"""

import contextlib
import numpy as np

# ---------------------------------------------------------------- constants
B, H, Wimg = 4, 192, 192
K = 9
F = 64
NCH = K * K  # 81
HO = H - K + 1  # 184
WO = Wimg - K + 1  # 184
UNF = 24
N_CORES = 8

ROWS = HO // 2  # 92 position rows per core
SLAB = ROWS + K - 1  # 100 image/output rows per core
NPOS = ROWS * WO  # 16928 positions per core
HALFR = ROWS // 2  # 46 rows per block-diag half
HALF = HALFR * WO  # 8464 columns per half

# iteration chunking: 512-col psum chunks, paired into 1024-col supersteps
CH = 512
CHUNKS = [(i * CH, min((i + 1) * CH, HALF)) for i in range((HALF + CH - 1) // CH)]
SUPERS = [CHUNKS[i: i + 2] for i in range(0, len(CHUNKS), 2)]

DCH = 2 * WO  # decode group = 2 position rows = 368 columns
NGRP = NPOS // DCH  # 46 groups (23 per half)
GPT = 2  # groups per decode psum tile (one per 512-col bank slot)

IMW_IM2COL = K * SLAB * WO  # 165600
IMW_TOT = IMW_IM2COL + SLAB * Wimg  # + raw slab [100,192]

# weight blob layout: (name, partitions, cols) — bf16
BLOB_SPEC = [
    ("wac", NCH, F), ("wacp", NCH, 128), ("sbd", 128, 128), ("nsbd", 128, 128),
    ("id128", 128, 128), ("wwa", 128, NCH), ("wwb", 128, NCH),
    ("eshb", ROWS, K * SLAB), ("bnd1", SLAB, ROWS), ("bnd2", ROWS, SLAB),
]
BLOBC = sum(nf for _, _, nf in BLOB_SPEC)

_STATE = {}


def _split_multi_waits(nc, mybir):
    """This walrus build supports a single sync-wait slot per instruction.
    Move extra waits onto preceding same-engine no-ops (same semantics:
    program order on one engine; all waits clear before the instruction)."""
    cnt = 0
    for fn in nc.m.functions:
        for bb in fn.blocks:
            insts = bb.instructions
            need = False
            for ins in insts:
                si = ins.sync_info
                if si is not None and si.on_wait is not None and len(si.on_wait) > 1:
                    need = True
                    break
            if not need:
                continue
            out = []
            for ins in insts:
                si = ins.sync_info
                if si is not None and si.on_wait is not None and len(si.on_wait) > 1:
                    waits = list(si.on_wait)
                    for w in waits[:-1]:
                        cnt += 1
                        nop = mybir.InstNoOp(name=f"wsplit-{cnt}", ins=[], outs=[])
                        nop.engine = ins.engine
                        nop.sync_info = mybir.SyncInfo(on_wait=[w], on_update=[])
                        out.append(nop)
                    ins.sync_info = mybir.SyncInfo(
                        on_wait=[waits[-1]], on_update=list(si.on_update or [])
                    )
                out.append(ins)
            bb.instructions = out
    return cnt


def _build():
    import concourse.bass as bass
    import concourse.mybir as mybir
    import concourse.tile as tile

    f32 = mybir.dt.float32
    bf16 = mybir.dt.bfloat16
    f8 = mybir.dt.float8e4
    Alu = mybir.AluOpType
    Act = mybir.ActivationFunctionType

    nc = bass.Bass("TRN2", target_bir_lowering=False, debug=False)

    imgw = nc.dram_tensor("imgw", [IMW_TOT], bf16, kind="ExternalInput").ap()
    blob_d = nc.dram_tensor("blob", [128, BLOBC], bf16, kind="ExternalInput").ap()
    lams_d = nc.dram_tensor("lams", [128, 2], f32, kind="ExternalInput").ap()
    out_d = nc.dram_tensor("out", [SLAB, Wimg], f32, kind="ExternalOutput").ap()
    # row-range-split obuf tensors: first-half col2im gathers start
    # mid-decode instead of waiting for a full-queue drain at the end
    RSPL = [0, 48, ROWS]
    obufs = [nc.dram_tensor(f"obuf{p}", [NCH * (RSPL[p + 1] - RSPL[p]) * WO],
                            bf16, kind="Internal").ap() for p in range(2)]

    with tile.TileContext(nc) as tc:
        with contextlib.ExitStack() as ctx:
            wpool = ctx.enter_context(tc.tile_pool(name="w", bufs=1))
            big = ctx.enter_context(tc.tile_pool(name="big", bufs=1))
            pp = ctx.enter_context(tc.tile_pool(name="ps", bufs=4, space="PSUM"))
            stg = ctx.enter_context(tc.tile_pool(name="stg", bufs=12))

            blob = wpool.tile([128, BLOBC], bf16)
            lams = wpool.tile([128, 2], f32)
            o = {}
            col = 0
            for name, np_, nf in BLOB_SPEC:
                o[name] = (np_, col, nf)
                col += nf

            def bl(name):
                np_, c0, nf = o[name]
                return blob[0:np_, c0:c0 + nf]

            wac = bl("wac"); wacp = bl("wacp"); sbd = bl("sbd")
            nsbd = bl("nsbd")
            id128 = bl("id128"); wwa = bl("wwa"); wwb = bl("wwb")
            eshb = bl("eshb"); bnd1 = bl("bnd1"); bnd2 = bl("bnd2")
            id100 = blob[0:SLAB, o["id128"][1]:o["id128"][1] + SLAB]
            lam = lams[:, 0:1]
            nlam = lams[:, 1:2]

            # persistent SBUF state
            icol = big.tile([NCH, NPOS], bf16, tag="icol")
            c = big.tile([128, HALF], bf16)
            gam = big.tile([128, HALF], bf16)
            yt = big.tile([128, HALF], bf16)
            zt = big.tile([128, HALF], bf16)
            slab = big.tile([SLAB, Wimg], bf16)
            t2m = big.tile([SLAB, Wimg], bf16)  # mean-path output grid term
            ms = big.tile([SLAB, 1024], bf16)   # mean-path scratch columns
            s1 = ms[:, 0:184]
            mrow = ms[0:ROWS, 192:376]
            t1 = ms[:, 384:568]
            ta = ms[:, 576:760]
            tb = ms[:, 768:960]

            # ---- input DMAs: slab first (mean path), then weights, then icol
            nc.sync.dma_start(
                slab[:], bass.AP(imgw.tensor, IMW_IM2COL, [[Wimg, SLAB], [1, Wimg]]))
            nc.scalar.dma_start(blob[:], blob_d)
            nc.scalar.dma_start(lams[:], lams_d)
            # im2col: host supplies img_w[kw] = slab[:, kw:kw+WO]; each channel
            # (kh, kw) = img_w[kw][kh:kh+ROWS] is one contiguous run.
            # 8 pieces ordered so early position rows land first (encode overlap).
            engs = (nc.sync, nc.gpsimd, nc.scalar)
            ei = 0
            for q in range(2):
                for hh in range(2):
                    r0 = hh * HALFR + q * (HALFR // 2)
                    r1 = hh * HALFR + (q + 1) * (HALFR // 2)
                    engs[ei % 3].dma_start(
                        icol[:, r0 * WO:r1 * WO],
                        bass.AP(imgw.tensor, r0 * WO,
                                [[WO, K], [SLAB * WO, K], [1, (r1 - r0) * WO]]))
                    ei += 1

            # ---- mean pathway (separable 9x9 box filters), pre-encode window.
            # M2[y,x] = sum_{kh,kw valid} mean[y-kh, x-kw],
            # mean[r,w] = (1/81) sum_{kh,kw} slab[r+kh, w+kw]
            # (idle pre-encode window: serialization of these tiny matmuls is
            # harmless, and every read element is written at least once)
            M1 = pp.tile([128, 1024], f32, tag="ps")
            PC = M1[0:ROWS, 512:696]
            for kw in range(K):
                # S1[w] = sum_kw slab[w+kw]: fixed out window, sliding rhs
                nc.tensor.matmul(M1[0:SLAB, 0:WO], id100,
                                 slab[:, kw:kw + WO],
                                 start=(kw == 0), stop=(kw == K - 1))
            nc.scalar.copy(ta[:, 0:184], M1[0:SLAB, 0:184])
            nc.tensor.matmul(PC, bnd1, ta[:, 0:184], start=True, stop=True)
            nc.scalar.copy(mrow, PC)  # bnd1 carries the 1/81 scale
            M2 = pp.tile([128, 1024], f32, tag="ps")
            nc.tensor.matmul(M2[0:SLAB, 512:696], bnd2, mrow,
                             start=True, stop=True)
            nc.scalar.copy(t1, M2[0:SLAB, 512:696])
            for kw in range(K):
                nc.tensor.matmul(M2[0:SLAB, kw:kw + 184], id100, t1,
                                 start=(kw == 0), stop=(kw == K - 1))
            nc.scalar.copy(t2m[:], M2[0:SLAB, 0:192])

            # ---- encode: c = WAc @ I_col for both halves (B via col-tile 64)
            for si, sup in enumerate(SUPERS):
                ps = pp.tile([128, 1024], f32, tag="ps")
                c0s, c1s = sup[0][0], sup[-1][1]
                for jj, (c0, c1) in enumerate(sup):
                    n = c1 - c0
                    nc.tensor.matmul(ps[0:128, jj * CH: jj * CH + n], wacp,
                                     icol[:, HALF + c0: HALF + c1],
                                     start=True, stop=True)
                    nc.tensor.matmul(ps[0:F, jj * CH: jj * CH + n], wac,
                                     icol[:, c0:c1], start=True, stop=True)
                span = c1s - c0s
                nc.scalar.copy(c[:, c0s:c1s], ps[:, 0:span])
                # gamma0 = c - clip(c) directly after each superstep's c lands
                nc.vector.tensor_scalar(zt[:, c0s:c1s], c[:, c0s:c1s],
                                        lam, nlam, Alu.min, Alu.max)
                nc.vector.tensor_tensor(gam[:, c0s:c1s], c[:, c0s:c1s],
                                        zt[:, c0s:c1s], Alu.subtract)
            # scheme-Z state init: y_0 = c (zt already holds clip(c))
            nc.vector.tensor_copy(yt[:, 7 * 1024:HALF], c[:, 7 * 1024:HALF])

            # ---- 23 fused iterations: y = Id@c + S@gam; gam' = y - clip(y)
            # Columns >= ZC (supersteps 7-8) use (y, z=clip(y)) state instead
            # of gam: the PE adds a third pass (-S@z), which deletes the DVE
            # subtract there; the freed DVE capacity casts cols >= ACT_END so
            # ACT streams less. The last iteration reverts to gam everywhere
            # (decode reads gam). soft(y) = y - clip(y) makes this exact.
            ZC = 7 * 1024  # scheme-Z columns [7168:8464]
            ACT_END = ZC + CH  # ACT copies [0:7680], DVE casts [7680:8464]
            for _t in range(UNF - 1):
                lastit = _t == UNF - 2
                for sp in range(0, len(SUPERS), 2):
                    pair = SUPERS[sp:sp + 2]
                    p0 = pair[0][0][0]
                    off = 0
                    for sup in pair:
                        ps = pp.tile([128, 1024], f32, tag="ps")
                        c0s, c1s = sup[0][0], sup[-1][1]
                        for jj, (c0, c1) in enumerate(sup):
                            nc.tensor.matmul(ps[:, jj * CH: jj * CH + (c1 - c0)],
                                             id128, c[:, c0:c1],
                                             start=True, stop=False)
                        for jj, (c0, c1) in enumerate(sup):
                            if c0 >= ZC:
                                nc.tensor.matmul(
                                    ps[:, jj * CH: jj * CH + (c1 - c0)],
                                    sbd, yt[:, c0:c1], start=False, stop=False)
                                nc.tensor.matmul(
                                    ps[:, jj * CH: jj * CH + (c1 - c0)],
                                    nsbd, zt[:, c0:c1], start=False, stop=True)
                            else:
                                nc.tensor.matmul(
                                    ps[:, jj * CH: jj * CH + (c1 - c0)],
                                    sbd, gam[:, c0:c1], start=False, stop=True)
                        span = c1s - c0s
                        if c0s >= ACT_END:
                            nc.vector.tensor_copy(yt[:, c0s:c1s], ps[:, 0:span])
                        elif c1s > ACT_END:
                            na = ACT_END - c0s
                            nc.scalar.copy(yt[:, c0s:ACT_END], ps[:, 0:na])
                            nc.vector.tensor_copy(yt[:, ACT_END:c1s],
                                                  ps[:, na:span])
                        else:
                            nc.scalar.copy(yt[:, c0s:c1s], ps[:, 0:span])
                        off += span
                    if sp < 6:
                        nc.vector.tensor_scalar(zt[:, p0:p0 + off],
                                                yt[:, p0:p0 + off],
                                                lam, nlam, Alu.min, Alu.max)
                        nc.vector.tensor_tensor(gam[:, p0:p0 + off],
                                                yt[:, p0:p0 + off],
                                                zt[:, p0:p0 + off], Alu.subtract)
                    elif sp == 8:
                        # merged clip over [6144:8464]: the Z-region has no
                        # subtract and its z-state isn't read until late in
                        # the next iteration, so one wide TS replaces two
                        nc.vector.tensor_scalar(zt[:, 6144:HALF],
                                                yt[:, 6144:HALF],
                                                lam, nlam, Alu.min, Alu.max)
                        tte = HALF if lastit else ZC
                        nc.vector.tensor_tensor(gam[:, 6144:tte],
                                                yt[:, 6144:tte],
                                                zt[:, 6144:tte], Alu.subtract)
                # pad the psum-pool rotation to 12 fills/iteration so every
                # iteration starts at the same (measured-fastest) buffer
                # phase; the 1x1 matmuls cost ~60ns each of idle PE slack
                for _dmy in range(3):
                    dmy = pp.tile([128, 1024], f32, tag="ps")
                    nc.tensor.matmul(dmy[0:1, 0:1], id128[0:1, 0:1],
                                     c[0:1, 0:1], start=True, stop=True)

            # ---- decode: out_all' = WW@gam per half, stream to HBM (obuf).
            # 46 two-row groups, 2 per psum tile in 512-col bank slots; one
            # strided extraction per tile alternating ACT/DVE.
            ntile = (NGRP + GPT - 1) // GPT
            stall = big.tile([ROWS, NCH * WO], bf16, tag="icol")
            for t in range(ntile):
                g0 = t * GPT
                ng = min(GPT, NGRP - g0)
                ps = pp.tile([128, 1024], f32, tag="ps")
                for j in range(ng):
                    g = g0 + j
                    half, gl = g // (NGRP // 2), g % (NGRP // 2)
                    ww = wwb if half else wwa
                    cc = gl * DCH
                    nc.tensor.matmul(ps[0:NCH, j * 512:j * 512 + DCH], ww,
                                     gam[:, cc:cc + DCH], start=True, stop=True)
                yd = stg.tile([NCH, GPT * DCH], bf16, tag="yd")
                src = ps.rearrange("p (g x) -> p g x", g=2)[0:NCH, 0:ng, 0:DCH]
                dst = yd.rearrange("p (g x) -> p g x", g=GPT)[:, 0:ng, :]
                if t % 2 == 0:
                    nc.scalar.copy(dst, src)
                else:
                    nc.vector.tensor_copy(dst, src)
                # each group = 2 position rows; rows are globally contiguous
                r0 = 2 * g0
                nrows = 2 * ng
                part = 0 if r0 < RSPL[1] else 1
                ob, rb = obufs[part], r0 - RSPL[part]
                dmadst = bass.AP(ob.tensor, rb * NCH * WO,
                                 [[WO, NCH], [NCH * WO, nrows], [1, WO]])
                eng = (nc.sync, nc.gpsimd, nc.scalar)[t % 3]
                eng.dma_start(dmadst, yd[:, 0:ng * DCH])
                if r0 + nrows == RSPL[1]:
                    # first-half rows fully written: start their gathers now
                    for kh in range(K):
                        geng = (nc.sync, nc.gpsimd)[kh % 2]
                        geng.dma_start(
                            stall[0:RSPL[1], kh * K * WO:(kh + 1) * K * WO],
                            bass.AP(obufs[0].tensor, kh * K * WO,
                                    [[NCH * WO, RSPL[1]], [1, K * WO]]))

            # ---- col2im: contiguous gathers (reuse icol's SBUF slot), then
            # shift-matmuls into 4 rotating PSUM bank slots (no serialization),
            # merged with the mean term at the end.
            for kh in range(K):
                eng = (nc.sync, nc.gpsimd)[(kh + 1) % 2]
                eng.dma_start(
                    stall[RSPL[1]:ROWS, kh * K * WO:(kh + 1) * K * WO],
                    bass.AP(obufs[1].tensor, kh * K * WO,
                            [[NCH * WO, ROWS - RSPL[1]], [1, K * WO]]))
            opsA = pp.tile([128, 1024], f32, tag="ps")
            opsB = pp.tile([128, 1024], f32, tag="ps")
            tiles = [opsA, opsA, opsB, opsB]
            bases = [0, 512, 0, 512]
            started = [False] * 4
            nmm = [0] * 4
            for i in range(NCH):
                nmm[i % 4] += 1
            done = [0] * 4
            for kh in range(K):
                lhs = eshb[:, kh * SLAB:(kh + 1) * SLAB]
                for kw in range(K):
                    i = kh * K + kw
                    sl = i % 4
                    done[sl] += 1
                    nc.tensor.matmul(
                        tiles[sl][0:SLAB, bases[sl] + kw:bases[sl] + kw + WO],
                        lhs, stall[:, i * WO:(i + 1) * WO],
                        start=not started[sl], stop=(done[sl] == nmm[sl]))
                    started[sl] = True
            # merge 4 slots + mean term -> f32 out
            q0 = stg.tile([SLAB, Wimg], bf16, tag="q0")
            q1 = stg.tile([SLAB, Wimg], bf16, tag="q1")
            u0 = stg.tile([SLAB, Wimg], bf16, tag="u0")
            u1 = stg.tile([SLAB, Wimg], bf16, tag="u1")
            acc = stg.tile([SLAB, Wimg], f32, tag="acc")
            nc.scalar.copy(q0[:], opsA[0:SLAB, 0:192])
            nc.vector.tensor_copy(q1[:], opsA[0:SLAB, 512:704])
            nc.vector.tensor_tensor(u0[:], q0[:], opsB[0:SLAB, 0:192], Alu.add)
            nc.vector.tensor_tensor(u1[:], q1[:], opsB[0:SLAB, 512:704], Alu.add)
            nc.vector.tensor_tensor(u0[:], u0[:], u1[:], Alu.add)
            nc.vector.tensor_tensor(u1[:], u0[:], t2m[:], Alu.add)
            nc.vector.tensor_copy(acc[:], u1[:])
            nc.sync.dma_start(out_d, acc[:])

    _split_multi_waits(nc, mybir)
    return nc


def _get_nc():
    if "nc" not in _STATE:
        _STATE["nc"] = _build()
    return _STATE["nc"]


def _make_in_maps(I, WA, WD, WW, lmbda):
    import ml_dtypes  # noqa: F401
    I = np.ascontiguousarray(np.asarray(I, np.float32))
    WA = np.asarray(WA, np.float32)
    WD = np.asarray(WD, np.float32)
    WW = np.asarray(WW, np.float32)
    lam = np.asarray(lmbda, np.float32).reshape(F)
    assert I.shape == (B, 1, H, Wimg)

    WAc = (WA - WA.mean(axis=1, keepdims=True)).astype(np.float32)  # [64,81]
    S = (np.eye(F, dtype=np.float32) - WA @ WD).astype(np.float32)  # [64,64]
    sbd = np.zeros((128, 128), np.float32)
    sbd[0:F, 0:F] = S.T
    sbd[F:128, F:128] = S.T
    id128 = np.eye(128, dtype=np.float32)
    wacp = np.zeros((81, 128), np.float32)
    wacp[:, F:128] = WAc.T
    wwa = np.zeros((128, 81), np.float32)
    wwa[0:F, :] = WW.T
    wwb = np.zeros((128, 81), np.float32)
    wwb[F:128, :] = WW.T
    lam128 = np.concatenate([lam, lam]).reshape(128, 1).astype(np.float32)
    esh = np.zeros((ROWS, K * SLAB), np.float32)  # lhsT per kh: E[r, y]=1 iff y=r+kh
    for kh in range(K):
        for rr in range(ROWS):
            esh[rr, kh * SLAB + rr + kh] = 1.0
    bnd1 = np.zeros((SLAB, ROWS), np.float32)  # S2[r] = sum_{p=r..r+8} S1[p] / 81
    for p in range(SLAB):
        for r in range(ROWS):
            if r <= p <= r + 8:
                bnd1[p, r] = 1.0 / NCH
    bnd2 = np.zeros((ROWS, SLAB), np.float32)  # T1[y] = sum_{r=y-8..y} m[r]
    for r in range(ROWS):
        for y in range(SLAB):
            if y - 8 <= r <= y:
                bnd2[r, y] = 1.0
    vals = {"wac": WAc.T, "wacp": wacp, "sbd": sbd, "nsbd": -sbd,
            "id128": id128, "wwa": wwa, "wwb": wwb, "eshb": esh,
            "bnd1": bnd1, "bnd2": bnd2}
    blob = np.zeros((128, BLOBC), np.float32)
    col = 0
    for name, np_, nf in BLOB_SPEC:
        v = np.asarray(vals[name], np.float32)
        assert v.shape == (np_, nf), (name, v.shape)
        blob[0:np_, col:col + nf] = v
        col += nf
    lams = np.concatenate([lam128, -lam128], axis=1).astype(np.float32)

    shared = {"blob": blob.astype(ml_dtypes.bfloat16), "lams": lams}
    in_maps = []
    for core in range(N_CORES):
        b, h = core // 2, core % 2
        r0 = h * ROWS
        slab = I[b, 0, r0:r0 + SLAB, :]
        imgw = np.stack([slab[:, kw:kw + WO] for kw in range(K)], axis=0)
        full = np.concatenate([
            np.ascontiguousarray(imgw).reshape(-1),
            np.ascontiguousarray(slab).reshape(-1)])
        in_maps.append({"imgw": full.astype(ml_dtypes.bfloat16), **shared})
    return in_maps


def _unshard(results):
    cnt = np.zeros((H, Wimg), np.float32)
    for kh in range(K):
        for kw in range(K):
            cnt[kh:kh + HO, kw:kw + WO] += 1.0
    out = np.zeros((B, 1, H, Wimg), np.float32)
    for b in range(B):
        acc = np.zeros((H, Wimg), np.float32)
        acc[0:SLAB, :] += results[2 * b]["out"]
        acc[ROWS:ROWS + SLAB, :] += results[2 * b + 1]["out"]
        out[b, 0] = acc / cnt
    return out


def kernel(I, WA, WD, WW, lmbda, kernel_size=9, stride=1, unfoldings=24, **_kw):
    from concourse import bass_utils

    assert int(kernel_size) == K and int(stride) == 1 and int(unfoldings) == UNF
    in_maps = _make_in_maps(I, WA, WD, WW, lmbda)
    nc = _get_nc()
    last = None
    for _attempt in range(3):
        try:
            res = bass_utils.run_bass_kernel_spmd(
                nc, in_maps, core_ids=list(range(N_CORES)))
            return _unshard(res.results)
        except Exception as e:  # transient NRT device errors: retry
            last = e
    raise last


# revision 80
# speedup vs baseline: 1.0208x; 1.0092x over previous
"""LISTA (learned ISTA) sparse-coding forward pass on 8 Trainium2 NeuronCores.

Problem: I [4,1,192,192] -> im2col(9x9) -> 24 soft-thresholded iterations over
64 filters -> decode -> col2im overlap-add average -> [4,1,192,192].

Sharding: 8 cores = 4 images x 2 position-row halves (92 rows of 184 positions
each). Each core computes its full LISTA pipeline plus the col2im partial sums
for its 100-row output slab; the host merges the 8-row seams between the two
slabs of each image and divides by the overlap counts (pure unshard glue).

Algebra used (exact rewrites of the reference up to fp assoc.):
  - mean-subtraction folded into encoder:  c = WAc @ I_col,
      WAc = WA - rowmean(WA)  (since mean_patch = (1/81) * ones^T I_col)
  - iteration fused:  gamma_{t+1} = soft(S @ gamma_t + c),  S = I - WA@WD
  - mean add-back is separable: col2im_avg(mean x ones81) = 9-tap box filters
    (rows then cols) applied twice to the input slab; computed with a handful
    of tiny matmuls in the otherwise-idle pre-encode window, added at the end.
  - decode therefore only needs out_all' = WW @ gamma (one stationary / half).

Per-iteration structure (the hot loop, per core: 128 partitions = 2 halves x
64 filters, 8464 position columns):
  PE:  y = Id@c + Sbd@gam into PSUM in five supersteps (4x2048 + 272)
  ACT: copies supersteps 0-3 PSUM->SBUF (4 big ACTIVATEs)
  DVE: casts the 272 tail, then clip (tensor_scalar 4x) + subtract
       (tensor_tensor 2x) to form gam = soft(y).
col2im uses 4 rotating PSUM accumulator slots so its 81 shift-matmuls
pipeline instead of serializing on one bank.
"""

import contextlib
import numpy as np

# ---------------------------------------------------------------- constants
B, H, Wimg = 4, 192, 192
K = 9
F = 64
NCH = K * K  # 81
HO = H - K + 1  # 184
WO = Wimg - K + 1  # 184
UNF = 24
N_CORES = 8

ROWS = HO // 2  # 92 position rows per core
SLAB = ROWS + K - 1  # 100 image/output rows per core
NPOS = ROWS * WO  # 16928 positions per core
HALFR = ROWS // 2  # 46 rows per block-diag half
HALF = HALFR * WO  # 8464 columns per half

# iteration chunking: 512-col psum chunks, paired into 1024-col supersteps
CH = 512
CHUNKS = [(i * CH, min((i + 1) * CH, HALF)) for i in range((HALF + CH - 1) // CH)]
SUPERS = [CHUNKS[i: i + 2] for i in range(0, len(CHUNKS), 2)]

DCH = 2 * WO  # decode group = 2 position rows = 368 columns
NGRP = NPOS // DCH  # 46 groups (23 per half)
GPT = 2  # groups per decode psum tile (one per 512-col bank slot)

IMW_IM2COL = K * SLAB * WO  # 165600
IMW_TOT = IMW_IM2COL + SLAB * Wimg  # + raw slab [100,192]

# weight blob layout: (name, partitions, cols) — bf16
BLOB_SPEC = [
    ("wac", NCH, F), ("wacp", NCH, 128), ("sbd", 128, 128), ("nsbd", 128, 128),
    ("id128", 128, 128), ("wwa", 128, NCH), ("wwb", 128, NCH),
    ("eshb", ROWS, K * SLAB), ("bnd1", SLAB, ROWS), ("bnd2", ROWS, SLAB),
]
BLOBC = sum(nf for _, _, nf in BLOB_SPEC)

_STATE = {}


def _split_multi_waits(nc, mybir):
    """This walrus build supports a single sync-wait slot per instruction.
    Move extra waits onto preceding same-engine no-ops (same semantics:
    program order on one engine; all waits clear before the instruction)."""
    cnt = 0
    for fn in nc.m.functions:
        for bb in fn.blocks:
            insts = bb.instructions
            need = False
            for ins in insts:
                si = ins.sync_info
                if si is not None and si.on_wait is not None and len(si.on_wait) > 1:
                    need = True
                    break
            if not need:
                continue
            out = []
            for ins in insts:
                si = ins.sync_info
                if si is not None and si.on_wait is not None and len(si.on_wait) > 1:
                    waits = list(si.on_wait)
                    for w in waits[:-1]:
                        cnt += 1
                        nop = mybir.InstNoOp(name=f"wsplit-{cnt}", ins=[], outs=[])
                        nop.engine = ins.engine
                        nop.sync_info = mybir.SyncInfo(on_wait=[w], on_update=[])
                        out.append(nop)
                    ins.sync_info = mybir.SyncInfo(
                        on_wait=[waits[-1]], on_update=list(si.on_update or [])
                    )
                out.append(ins)
            bb.instructions = out
    return cnt


def _build():
    import concourse.bass as bass
    import concourse.mybir as mybir
    import concourse.tile as tile

    f32 = mybir.dt.float32
    bf16 = mybir.dt.bfloat16
    f8 = mybir.dt.float8e4
    Alu = mybir.AluOpType
    Act = mybir.ActivationFunctionType

    nc = bass.Bass("TRN2", target_bir_lowering=False, debug=False)

    imgw = nc.dram_tensor("imgw", [IMW_TOT], bf16, kind="ExternalInput").ap()
    blob_d = nc.dram_tensor("blob", [128, BLOBC], bf16, kind="ExternalInput").ap()
    lams_d = nc.dram_tensor("lams", [128, 2], f32, kind="ExternalInput").ap()
    out_d = nc.dram_tensor("out", [SLAB, Wimg], f32, kind="ExternalOutput").ap()
    # row-range-split obuf tensors: first-half col2im gathers start
    # mid-decode instead of waiting for a full-queue drain at the end
    RSPL = [0, 48, ROWS]
    obufs = [nc.dram_tensor(f"obuf{p}", [NCH * (RSPL[p + 1] - RSPL[p]) * WO],
                            bf16, kind="Internal").ap() for p in range(2)]

    with tile.TileContext(nc) as tc:
        with contextlib.ExitStack() as ctx:
            wpool = ctx.enter_context(tc.tile_pool(name="w", bufs=1))
            big = ctx.enter_context(tc.tile_pool(name="big", bufs=1))
            pp = ctx.enter_context(tc.tile_pool(name="ps", bufs=4, space="PSUM"))
            stg = ctx.enter_context(tc.tile_pool(name="stg", bufs=12))

            blob = wpool.tile([128, BLOBC], bf16)
            lams = wpool.tile([128, 2], f32)
            o = {}
            col = 0
            for name, np_, nf in BLOB_SPEC:
                o[name] = (np_, col, nf)
                col += nf

            def bl(name):
                np_, c0, nf = o[name]
                return blob[0:np_, c0:c0 + nf]

            wac = bl("wac"); wacp = bl("wacp"); sbd = bl("sbd")
            nsbd = bl("nsbd")
            id128 = bl("id128"); wwa = bl("wwa"); wwb = bl("wwb")
            eshb = bl("eshb"); bnd1 = bl("bnd1"); bnd2 = bl("bnd2")
            id100 = blob[0:SLAB, o["id128"][1]:o["id128"][1] + SLAB]
            lam = lams[:, 0:1]
            nlam = lams[:, 1:2]

            # persistent SBUF state
            icol = big.tile([NCH, NPOS], bf16, tag="icol")
            c = big.tile([128, HALF], bf16)
            gam = big.tile([128, HALF], bf16)
            yt = big.tile([128, HALF], bf16)
            zt = big.tile([128, HALF], bf16)
            slab = big.tile([SLAB, Wimg], bf16)
            t2m = big.tile([SLAB, Wimg], bf16)  # mean-path output grid term
            ms = big.tile([SLAB, 1024], bf16)   # mean-path scratch columns
            s1 = ms[:, 0:184]
            mrow = ms[0:ROWS, 192:376]
            t1 = ms[:, 384:568]
            ta = ms[:, 576:760]
            tb = ms[:, 768:960]

            # ---- input DMAs: slab first (mean path), then weights, then icol
            nc.sync.dma_start(
                slab[:], bass.AP(imgw.tensor, IMW_IM2COL, [[Wimg, SLAB], [1, Wimg]]))
            nc.scalar.dma_start(blob[:], blob_d)
            nc.scalar.dma_start(lams[:], lams_d)
            # im2col: host supplies img_w[kw] = slab[:, kw:kw+WO]; each channel
            # (kh, kw) = img_w[kw][kh:kh+ROWS] is one contiguous run.
            # 8 pieces ordered so early position rows land first (encode overlap).
            engs = (nc.sync, nc.gpsimd, nc.scalar)
            ei = 0
            for q in range(2):
                for hh in range(2):
                    r0 = hh * HALFR + q * (HALFR // 2)
                    r1 = hh * HALFR + (q + 1) * (HALFR // 2)
                    engs[ei % 3].dma_start(
                        icol[:, r0 * WO:r1 * WO],
                        bass.AP(imgw.tensor, r0 * WO,
                                [[WO, K], [SLAB * WO, K], [1, (r1 - r0) * WO]]))
                    ei += 1

            # ---- mean pathway (separable 9x9 box filters), pre-encode window.
            # M2[y,x] = sum_{kh,kw valid} mean[y-kh, x-kw],
            # mean[r,w] = (1/81) sum_{kh,kw} slab[r+kh, w+kw]
            # (idle pre-encode window: serialization of these tiny matmuls is
            # harmless, and every read element is written at least once)
            M1 = pp.tile([128, 1024], f32, tag="ps")
            PC = M1[0:ROWS, 512:696]
            for kw in range(K):
                # S1[w] = sum_kw slab[w+kw]: fixed out window, sliding rhs
                nc.tensor.matmul(M1[0:SLAB, 0:WO], id100,
                                 slab[:, kw:kw + WO],
                                 start=(kw == 0), stop=(kw == K - 1))
            nc.scalar.copy(ta[:, 0:184], M1[0:SLAB, 0:184])
            nc.tensor.matmul(PC, bnd1, ta[:, 0:184], start=True, stop=True)
            nc.scalar.copy(mrow, PC)  # bnd1 carries the 1/81 scale
            M2 = pp.tile([128, 1024], f32, tag="ps")
            nc.tensor.matmul(M2[0:SLAB, 512:696], bnd2, mrow,
                             start=True, stop=True)
            nc.scalar.copy(t1, M2[0:SLAB, 512:696])
            for kw in range(K):
                nc.tensor.matmul(M2[0:SLAB, kw:kw + 184], id100, t1,
                                 start=(kw == 0), stop=(kw == K - 1))
            nc.scalar.copy(t2m[:], M2[0:SLAB, 0:192])

            # ---- encode: c = WAc @ I_col for both halves (B via col-tile 64)
            for si, sup in enumerate(SUPERS):
                ps = pp.tile([128, 1024], f32, tag="ps")
                c0s, c1s = sup[0][0], sup[-1][1]
                for jj, (c0, c1) in enumerate(sup):
                    n = c1 - c0
                    nc.tensor.matmul(ps[0:128, jj * CH: jj * CH + n], wacp,
                                     icol[:, HALF + c0: HALF + c1],
                                     start=True, stop=True)
                    nc.tensor.matmul(ps[0:F, jj * CH: jj * CH + n], wac,
                                     icol[:, c0:c1], start=True, stop=True)
                span = c1s - c0s
                nc.scalar.copy(c[:, c0s:c1s], ps[:, 0:span])
                # gamma0 = c - clip(c) directly after each superstep's c lands
                nc.vector.tensor_scalar(zt[:, c0s:c1s], c[:, c0s:c1s],
                                        lam, nlam, Alu.min, Alu.max)
                nc.vector.tensor_tensor(gam[:, c0s:c1s], c[:, c0s:c1s],
                                        zt[:, c0s:c1s], Alu.subtract)
            # scheme-Z state init: y_0 = c (zt already holds clip(c))
            nc.vector.tensor_copy(yt[:, 7 * 1024:HALF], c[:, 7 * 1024:HALF])

            # ---- 23 fused iterations: y = Id@c + S@gam; gam' = y - clip(y)
            # Columns >= ZC (supersteps 7-8) use (y, z=clip(y)) state instead
            # of gam: the PE adds a third pass (-S@z), which deletes the DVE
            # subtract there; the freed DVE capacity casts cols >= ACT_END so
            # ACT streams less. The last iteration reverts to gam everywhere
            # (decode reads gam). soft(y) = y - clip(y) makes this exact.
            ZC = 7 * 1024  # scheme-Z columns [7168:8464]
            ACT_END = ZC + CH  # ACT copies [0:7680], DVE casts [7680:8464]
            for _t in range(UNF - 1):
                lastit = _t == UNF - 2
                for sp in range(0, len(SUPERS), 2):
                    pair = SUPERS[sp:sp + 2]
                    p0 = pair[0][0][0]
                    off = 0
                    for sup in pair:
                        ps = pp.tile([128, 1024], f32, tag="ps")
                        c0s, c1s = sup[0][0], sup[-1][1]
                        for jj, (c0, c1) in enumerate(sup):
                            nc.tensor.matmul(ps[:, jj * CH: jj * CH + (c1 - c0)],
                                             id128, c[:, c0:c1],
                                             start=True, stop=False)
                        for jj, (c0, c1) in enumerate(sup):
                            if c0 >= ZC:
                                nc.tensor.matmul(
                                    ps[:, jj * CH: jj * CH + (c1 - c0)],
                                    sbd, yt[:, c0:c1], start=False, stop=False)
                            else:
                                nc.tensor.matmul(
                                    ps[:, jj * CH: jj * CH + (c1 - c0)],
                                    sbd, gam[:, c0:c1], start=False, stop=True)
                        # nsbd passes grouped last: one stationary swap
                        # instead of two per Z-superstep on the critical path
                        for jj, (c0, c1) in enumerate(sup):
                            if c0 >= ZC:
                                nc.tensor.matmul(
                                    ps[:, jj * CH: jj * CH + (c1 - c0)],
                                    nsbd, zt[:, c0:c1], start=False, stop=True)
                        span = c1s - c0s
                        if c0s >= ACT_END:
                            nc.vector.tensor_copy(yt[:, c0s:c1s], ps[:, 0:span])
                        elif c1s > ACT_END:
                            na = ACT_END - c0s
                            nc.scalar.copy(yt[:, c0s:ACT_END], ps[:, 0:na])
                            nc.vector.tensor_copy(yt[:, ACT_END:c1s],
                                                  ps[:, na:span])
                        else:
                            nc.scalar.copy(yt[:, c0s:c1s], ps[:, 0:span])
                        off += span
                    if sp < 6:
                        nc.vector.tensor_scalar(zt[:, p0:p0 + off],
                                                yt[:, p0:p0 + off],
                                                lam, nlam, Alu.min, Alu.max)
                        nc.vector.tensor_tensor(gam[:, p0:p0 + off],
                                                yt[:, p0:p0 + off],
                                                zt[:, p0:p0 + off], Alu.subtract)
                    elif sp == 8:
                        # merged clip over [6144:8464]: the Z-region has no
                        # subtract and its z-state isn't read until late in
                        # the next iteration, so one wide TS replaces two
                        nc.vector.tensor_scalar(zt[:, 6144:HALF],
                                                yt[:, 6144:HALF],
                                                lam, nlam, Alu.min, Alu.max)
                        tte = HALF if lastit else ZC
                        nc.vector.tensor_tensor(gam[:, 6144:tte],
                                                yt[:, 6144:tte],
                                                zt[:, 6144:tte], Alu.subtract)
                # pad the psum-pool rotation to 12 fills/iteration so every
                # iteration starts at the same (measured-fastest) buffer
                # phase; the 1x1 matmuls cost ~60ns each of idle PE slack
                for _dmy in range(3):
                    dmy = pp.tile([128, 1024], f32, tag="ps")
                    nc.tensor.matmul(dmy[0:1, 0:1], id128[0:1, 0:1],
                                     c[0:1, 0:1], start=True, stop=True)

            # ---- decode: out_all' = WW@gam per half, stream to HBM (obuf).
            # 46 two-row groups, 2 per psum tile in 512-col bank slots; one
            # strided extraction per tile alternating ACT/DVE.
            ntile = (NGRP + GPT - 1) // GPT
            stall = big.tile([ROWS, NCH * WO], bf16, tag="icol")
            for t in range(ntile):
                g0 = t * GPT
                ng = min(GPT, NGRP - g0)
                ps = pp.tile([128, 1024], f32, tag="ps")
                for j in range(ng):
                    g = g0 + j
                    half, gl = g // (NGRP // 2), g % (NGRP // 2)
                    ww = wwb if half else wwa
                    cc = gl * DCH
                    nc.tensor.matmul(ps[0:NCH, j * 512:j * 512 + DCH], ww,
                                     gam[:, cc:cc + DCH], start=True, stop=True)
                yd = stg.tile([NCH, GPT * DCH], bf16, tag="yd")
                src = ps.rearrange("p (g x) -> p g x", g=2)[0:NCH, 0:ng, 0:DCH]
                dst = yd.rearrange("p (g x) -> p g x", g=GPT)[:, 0:ng, :]
                if t % 2 == 0:
                    nc.scalar.copy(dst, src)
                else:
                    nc.vector.tensor_copy(dst, src)
                # each group = 2 position rows; rows are globally contiguous
                r0 = 2 * g0
                nrows = 2 * ng
                part = 0 if r0 < RSPL[1] else 1
                ob, rb = obufs[part], r0 - RSPL[part]
                dmadst = bass.AP(ob.tensor, rb * NCH * WO,
                                 [[WO, NCH], [NCH * WO, nrows], [1, WO]])
                eng = (nc.sync, nc.gpsimd, nc.scalar)[t % 3]
                eng.dma_start(dmadst, yd[:, 0:ng * DCH])
                if r0 + nrows == RSPL[1]:
                    # first-half rows fully written: start their gathers now
                    for kh in range(K):
                        geng = (nc.sync, nc.gpsimd)[kh % 2]
                        geng.dma_start(
                            stall[0:RSPL[1], kh * K * WO:(kh + 1) * K * WO],
                            bass.AP(obufs[0].tensor, kh * K * WO,
                                    [[NCH * WO, RSPL[1]], [1, K * WO]]))

            # ---- col2im: contiguous gathers (reuse icol's SBUF slot), then
            # shift-matmuls into 4 rotating PSUM bank slots (no serialization),
            # merged with the mean term at the end.
            for kh in range(K):
                eng = (nc.sync, nc.gpsimd)[(kh + 1) % 2]
                eng.dma_start(
                    stall[RSPL[1]:ROWS, kh * K * WO:(kh + 1) * K * WO],
                    bass.AP(obufs[1].tensor, kh * K * WO,
                            [[NCH * WO, ROWS - RSPL[1]], [1, K * WO]]))
            opsA = pp.tile([128, 1024], f32, tag="ps")
            opsB = pp.tile([128, 1024], f32, tag="ps")
            tiles = [opsA, opsA, opsB, opsB]
            bases = [0, 512, 0, 512]
            started = [False] * 4
            nmm = [0] * 4
            for i in range(NCH):
                nmm[i % 4] += 1
            done = [0] * 4
            for kh in range(K):
                lhs = eshb[:, kh * SLAB:(kh + 1) * SLAB]
                for kw in range(K):
                    i = kh * K + kw
                    sl = i % 4
                    done[sl] += 1
                    nc.tensor.matmul(
                        tiles[sl][0:SLAB, bases[sl] + kw:bases[sl] + kw + WO],
                        lhs, stall[:, i * WO:(i + 1) * WO],
                        start=not started[sl], stop=(done[sl] == nmm[sl]))
                    started[sl] = True
            # merge 4 slots + mean term -> f32 out
            q0 = stg.tile([SLAB, Wimg], bf16, tag="q0")
            q1 = stg.tile([SLAB, Wimg], bf16, tag="q1")
            u0 = stg.tile([SLAB, Wimg], bf16, tag="u0")
            u1 = stg.tile([SLAB, Wimg], bf16, tag="u1")
            acc = stg.tile([SLAB, Wimg], f32, tag="acc")
            nc.scalar.copy(q0[:], opsA[0:SLAB, 0:192])
            nc.vector.tensor_copy(q1[:], opsA[0:SLAB, 512:704])
            nc.vector.tensor_tensor(u0[:], q0[:], opsB[0:SLAB, 0:192], Alu.add)
            nc.vector.tensor_tensor(u1[:], q1[:], opsB[0:SLAB, 512:704], Alu.add)
            nc.vector.tensor_tensor(u0[:], u0[:], u1[:], Alu.add)
            nc.vector.tensor_tensor(u1[:], u0[:], t2m[:], Alu.add)
            nc.vector.tensor_copy(acc[:], u1[:])
            nc.sync.dma_start(out_d, acc[:])

    _split_multi_waits(nc, mybir)
    return nc


def _get_nc():
    if "nc" not in _STATE:
        _STATE["nc"] = _build()
    return _STATE["nc"]


def _make_in_maps(I, WA, WD, WW, lmbda):
    import ml_dtypes  # noqa: F401
    I = np.ascontiguousarray(np.asarray(I, np.float32))
    WA = np.asarray(WA, np.float32)
    WD = np.asarray(WD, np.float32)
    WW = np.asarray(WW, np.float32)
    lam = np.asarray(lmbda, np.float32).reshape(F)
    assert I.shape == (B, 1, H, Wimg)

    WAc = (WA - WA.mean(axis=1, keepdims=True)).astype(np.float32)  # [64,81]
    S = (np.eye(F, dtype=np.float32) - WA @ WD).astype(np.float32)  # [64,64]
    sbd = np.zeros((128, 128), np.float32)
    sbd[0:F, 0:F] = S.T
    sbd[F:128, F:128] = S.T
    id128 = np.eye(128, dtype=np.float32)
    wacp = np.zeros((81, 128), np.float32)
    wacp[:, F:128] = WAc.T
    wwa = np.zeros((128, 81), np.float32)
    wwa[0:F, :] = WW.T
    wwb = np.zeros((128, 81), np.float32)
    wwb[F:128, :] = WW.T
    lam128 = np.concatenate([lam, lam]).reshape(128, 1).astype(np.float32)
    esh = np.zeros((ROWS, K * SLAB), np.float32)  # lhsT per kh: E[r, y]=1 iff y=r+kh
    for kh in range(K):
        for rr in range(ROWS):
            esh[rr, kh * SLAB + rr + kh] = 1.0
    bnd1 = np.zeros((SLAB, ROWS), np.float32)  # S2[r] = sum_{p=r..r+8} S1[p] / 81
    for p in range(SLAB):
        for r in range(ROWS):
            if r <= p <= r + 8:
                bnd1[p, r] = 1.0 / NCH
    bnd2 = np.zeros((ROWS, SLAB), np.float32)  # T1[y] = sum_{r=y-8..y} m[r]
    for r in range(ROWS):
        for y in range(SLAB):
            if y - 8 <= r <= y:
                bnd2[r, y] = 1.0
    vals = {"wac": WAc.T, "wacp": wacp, "sbd": sbd, "nsbd": -sbd,
            "id128": id128, "wwa": wwa, "wwb": wwb, "eshb": esh,
            "bnd1": bnd1, "bnd2": bnd2}
    blob = np.zeros((128, BLOBC), np.float32)
    col = 0
    for name, np_, nf in BLOB_SPEC:
        v = np.asarray(vals[name], np.float32)
        assert v.shape == (np_, nf), (name, v.shape)
        blob[0:np_, col:col + nf] = v
        col += nf
    lams = np.concatenate([lam128, -lam128], axis=1).astype(np.float32)

    shared = {"blob": blob.astype(ml_dtypes.bfloat16), "lams": lams}
    in_maps = []
    for core in range(N_CORES):
        b, h = core // 2, core % 2
        r0 = h * ROWS
        slab = I[b, 0, r0:r0 + SLAB, :]
        imgw = np.stack([slab[:, kw:kw + WO] for kw in range(K)], axis=0)
        full = np.concatenate([
            np.ascontiguousarray(imgw).reshape(-1),
            np.ascontiguousarray(slab).reshape(-1)])
        in_maps.append({"imgw": full.astype(ml_dtypes.bfloat16), **shared})
    return in_maps


def _unshard(results):
    cnt = np.zeros((H, Wimg), np.float32)
    for kh in range(K):
        for kw in range(K):
            cnt[kh:kh + HO, kw:kw + WO] += 1.0
    out = np.zeros((B, 1, H, Wimg), np.float32)
    for b in range(B):
        acc = np.zeros((H, Wimg), np.float32)
        acc[0:SLAB, :] += results[2 * b]["out"]
        acc[ROWS:ROWS + SLAB, :] += results[2 * b + 1]["out"]
        out[b, 0] = acc / cnt
    return out


def kernel(I, WA, WD, WW, lmbda, kernel_size=9, stride=1, unfoldings=24, **_kw):
    from concourse import bass_utils

    assert int(kernel_size) == K and int(stride) == 1 and int(unfoldings) == UNF
    in_maps = _make_in_maps(I, WA, WD, WW, lmbda)
    nc = _get_nc()
    last = None
    for _attempt in range(3):
        try:
            res = bass_utils.run_bass_kernel_spmd(
                nc, in_maps, core_ids=list(range(N_CORES)))
            return _unshard(res.results)
        except Exception as e:  # transient NRT device errors: retry
            last = e
    raise last
